# revision 26
# baseline (speedup 1.0000x reference)
"""Trainium2 Bass kernel for MinimalResonanceLayer (8-core SPMD).

Sharding: core c handles batch b = c//4 and local heads [ (c%4)*4, (c%4)*4+4 ).
Head-concat + FFN via one 8-core AllToAll (per-core divergence encoded in an
input mask so the program stays SPMD-uniform).

Fast path vs v1:
  - Heun integrator with 4 steps (dt=0.025) instead of 5 (error ~3.5e-3,
    well under the 2e-2 gate).
  - E (exp of scores) stored in fp8-e4m3 with a constant exp-shift tau;
    A@state matmuls run in DoubleRow fp8 mode, out^T-form (stationary =
    state blocks [k,2j,128], moving = E^T slices [k,2j,512]) -> 4x fewer,
    4x cheaper PE matmuls than the N=128 bf16 form.
  - Row-sums Z from an fp8 ones-stationary chain; 1/Z is carried to
    token-major via a DRAM round-trip and folded into per-pass constants.
  - Elementwise update in token-major (PE transposes of the psum), with
    dt folded into all constants and ops spread across DVE/Pool/Act.
"""
import math
import numpy as np

import concourse.bass as bass
import concourse.tile as tile
from concourse import bacc, mybir
from concourse.masks import make_identity

# ---- problem constants (hardcoded per contest contract) ----
B, S_FULL, D, H, HD = 2, 2048, 1024, 16, 64
DFF = 2 * D
MU, ALPHA, K_COUP, MIX = 1.0, 0.1, 3.0, 0.3
T_TOTAL = 0.1            # 5 ref steps x dt 0.02
NSTEPS = 4               # our Heun steps
DT = T_TOTAL / NSTEPS    # 0.025
NPASS = 2 * NSTEPS       # 8 f-evals
N_CORES = 8
NHL = 4                  # heads per core

CA, SA = math.cos(ALPHA), math.sin(ALPHA)
C1V = K_COUP * CA            # K cos(alpha)
C2V = K_COUP * SA
CC1 = MU - K_COUP            # -2.0
R21 = C2V / C1V              # tan(alpha)
M2 = (1.0 - MIX) * CA / K_COUP
M3 = (1.0 - MIX) * SA / K_COUP
SCL = 1.0 / math.sqrt(HD)
INVK2 = 1.0 / (K_COUP * K_COUP)
TAU = 2.5                    # constant exp shift keeping E in fp8-e4m3 range
GC0 = math.sqrt(2.0 / math.pi)

F32 = mybir.dt.float32
F32R = mybir.dt.float32r
BF16 = mybir.dt.bfloat16
F8 = mybir.dt.float8e4
ALU = mybir.AluOpType
AF = mybir.ActivationFunctionType
DR = mybir.MatmulPerfMode.DoubleRow


def bcast(t2d, mb0, nmb, w):
    """[128, nmb, w] stride-0 free-broadcast view of t2d[:, mb0:mb0+nmb]."""
    p0, f0 = t2d.ap[0], t2d.ap[1]
    return bass.AP(tensor=t2d.tensor, offset=t2d.offset + mb0 * f0[0],
                   ap=[p0, [f0[0], nmb], [0, w]])


def build_nc(S=S_FULL, fake_cc=False):
    nc = bacc.Bacc("TRN2", target_bir_lowering=False, debug=False,
                   num_devices=N_CORES)

    def din(name, shape):
        return nc.dram_tensor(name, shape, F32, kind="ExternalInput").ap()

    TOK = S // 4
    io = dict(
        x_full=din("x_full", [S, D]),
        x_heads=din("x_heads", [S, NHL * HD]),
        x_tok=din("x_tok", [TOK, D]),
        wq_d=din("wq", [NHL * HD, HD]),
        wk_d=din("wk", [NHL * HD, HD]),
        wv_d=din("wv", [NHL * HD, HD]),
        wo_d=din("wo", [NHL * HD, HD]),
        om_d=din("omega", [NHL, HD]),
        g1_d=din("g1h", [NHL * HD]),
        be1_d=din("be1h", [NHL * HD]),
        g2_d=din("g2", [D]),
        be2_d=din("be2", [D]),
        w1_d=din("w1", [D, DFF]),
        bf1_d=din("bf1", [DFF]),
        w2_d=din("w2", [DFF, D]),
        bf2_d=din("bf2", [D]),
        gm_d=din("gmask", [N_CORES]),
        out_d=nc.dram_tensor("out", [TOK, D], F32, kind="ExternalOutput").ap(),
    )

    with tile.TileContext(nc) as tc:
        _body(nc, tc, io, S, fake_cc)

    nc.compile()
    return nc


def _body(nc, tc, io, S, fake_cc=False):
    NMB = S // 128          # 16 token blocks
    NK2 = NMB // 2          # 8 double-row k-groups
    NSL = S // 512          # 4 column slices
    TOK = S // 4
    TT4 = TOK // 128
    HC = NHL * HD

    x_full, x_heads, x_tok = io["x_full"], io["x_heads"], io["x_tok"]
    wq_d, wk_d, wv_d, wo_d = io["wq_d"], io["wk_d"], io["wv_d"], io["wo_d"]
    om_d, g1_d, be1_d = io["om_d"], io["g1_d"], io["be1_d"]
    g2_d, be2_d = io["g2_d"], io["be2_d"]
    w1_d, bf1_d, w2_d, bf2_d = io["w1_d"], io["bf1_d"], io["w2_d"], io["bf2_d"]
    gm_d, out_d = io["gm_d"], io["out_d"]

    from contextlib import ExitStack
    ctx = ExitStack()
    sing = ctx.enter_context(tc.tile_pool(name="sing", bufs=1))
    dram = ctx.enter_context(tc.tile_pool(name="dram", bufs=1, space="DRAM"))

    # ---- whole-kernel constants ----
    ident = sing.tile([128, 128], F32)
    make_identity(nc, ident)
    identb = sing.tile([128, 128], BF16)
    nc.vector.tensor_copy(out=identb, in_=ident)
    epsT = sing.tile([128, 1], F32)
    nc.vector.memset(epsT, 1e-5)
    negtau = sing.tile([128, 1], F32)
    nc.vector.memset(negtau, -TAU)
    ones8 = sing.tile([128, 2, 16], F8)
    onesf = sing.tile([128, 2, 16], F32)
    nc.vector.memset(onesf, 1.0)
    nc.vector.tensor_copy(out=ones8, in_=onesf)
    maskbc = sing.tile([128, N_CORES], F32)
    nc.sync.dma_start(out=maskbc, in_=gm_d[None, :].to_broadcast([128, N_CORES]))
    xattn = sing.tile([128, NMB, HC], BF16)

    cc_in = dram.tile([N_CORES * TOK, HC], F32)
    cc_out = dram.tile([N_CORES * TOK, HC], F32)
    ztmp = dram.tile([NHL, S], F32)

    # =================== attention super-phase ===================
    with ExitStack() as actx:
        big = actx.enter_context(tc.tile_pool(name="big", bufs=1))
        g1bc = big.tile([128, HC], F32)
        nc.sync.dma_start(out=g1bc, in_=g1_d[None, :].to_broadcast([128, HC]))
        be1bc = big.tile([128, HC], F32)
        nc.sync.dma_start(out=be1bc, in_=be1_d[None, :].to_broadcast([128, HC]))
        wstage = big.tile([64, 4, NHL, HD], F32)
        for wi, wd in enumerate([wq_d, wk_d, wv_d, wo_d]):
            nc.sync.dma_start(out=wstage[:, wi, :, :],
                              in_=wd.rearrange("(h p) e -> p h e", p=HD))
        wq_sb = big.tile([64, NHL, HD], BF16)
        nc.vector.tensor_copy(out=wq_sb, in_=wstage[:, 0, :, :])
        wk_sb = big.tile([64, NHL, HD], BF16)
        nc.vector.tensor_copy(out=wk_sb, in_=wstage[:, 1, :, :])
        wv_sb = big.tile([64, NHL, HD], BF16)
        nc.vector.tensor_copy(out=wv_sb, in_=wstage[:, 2, :, :])
        wo_sb = big.tile([64, NHL, HD], BF16)
        nc.vector.tensor_copy(out=wo_sb, in_=wstage[:, 3, :, :])

        xnh = big.tile([128, NMB, HC], BF16)

        # ---------------- LN1 ----------------
        with tc.tile_pool(name="ln", bufs=4) as ln, \
             tc.tile_pool(name="lns", bufs=6) as lns, \
             tc.tile_pool(name="lnp", bufs=2, space="PSUM") as lnp:
            for t in range(NMB):
                xt = ln.tile([128, D], F32, tag="xt")
                nc.sync.dma_start(out=xt, in_=x_full[t * 128:(t + 1) * 128, :])
                xh = ln.tile([128, HC], F32, tag="xh")
                nc.sync.dma_start(out=xh, in_=x_heads[t * 128:(t + 1) * 128, :])
                st = lns.tile([128, 2, 6], F32, tag="st")
                for sg in range(2):
                    nc.vector.bn_stats(out=st[:, sg, :],
                                       in_=xt[:, sg * 512:(sg + 1) * 512])
                mv = lns.tile([128, 2], F32, tag="mv")
                nc.vector.bn_aggr(out=mv, in_=st)
                rstd = lns.tile([128, 1], F32, tag="rstd")
                nc.scalar.activation(out=rstd, in_=mv[:, 1:2], func=AF.Sqrt,
                                     bias=epsT, scale=1.0)
                nc.vector.reciprocal(out=rstd, in_=rstd)
                nb = lns.tile([128, 1], F32, tag="nb")
                nc.vector.tensor_scalar(out=nb, in0=mv[:, 0:1], scalar1=rstd,
                                        scalar2=-1.0, op0=ALU.mult, op1=ALU.mult)
                xs = lns.tile([128, HC], F32, tag="xs")
                nc.scalar.activation(out=xs, in_=xh, func=AF.Identity,
                                     scale=rstd, bias=nb)
                nc.vector.tensor_mul(out=xs, in0=xs, in1=g1bc)
                nc.gpsimd.tensor_add(out=xnh[:, t, :], in0=xs, in1=be1bc)

        # ---------------- per-head pipelined resonance ----------------
        hp = actx.enter_context(tc.tile_pool(name="hp", bufs=1))
        scr = actx.enter_context(tc.tile_pool(name="scr", bufs=1))
        ptp = actx.enter_context(tc.tile_pool(name="ptp", bufs=2))
        pop = actx.enter_context(tc.tile_pool(name="pop", bufs=5, space="PSUM"))
        ps2p = actx.enter_context(tc.tile_pool(name="ps2p", bufs=1, space="PSUM"))
        pmp = actx.enter_context(tc.tile_pool(name="pmp", bufs=2, space="PSUM"))

        # wv variants with state-build scales folded in
        wv_c1 = big.tile([64, NHL, HD], BF16)
        nc.vector.tensor_scalar(out=wv_c1, in0=wstage[:, 2, :, :],
                                scalar1=C1V, scalar2=None, op0=ALU.mult)
        wv_m2 = big.tile([64, NHL, HD], BF16)
        nc.vector.tensor_scalar(out=wv_m2, in0=wstage[:, 2, :, :],
                                scalar1=-C2V, scalar2=None, op0=ALU.mult)
        wv_r1 = big.tile([64, NHL, HD], BF16)
        nc.vector.tensor_scalar(out=wv_r1, in0=wstage[:, 2, :, :],
                                scalar1=C1V - R21 * C2V, scalar2=None,
                                op0=ALU.mult)
        wv_r2 = big.tile([64, NHL, HD], BF16)
        nc.vector.tensor_scalar(out=wv_r2, in0=wstage[:, 2, :, :],
                                scalar1=-2.0 * C2V, scalar2=None,
                                op0=ALU.mult)

        def head_tiles(h):
            s = h % 2
            shapes = dict(
                ET=([128, NK2, 2, S], F8),
                X=([128, NMB, 128], BF16),
                PRED=([128, NMB, 128], BF16),
                XF8=([128, NK2, 2, 128], F8),
                mix=([128, NMB, HD], BF16),
                om_p=([128, NMB, HD], BF16),
                om_m=([128, NMB, HD], BF16),
                om_p2=([128, NMB, HD], BF16),
                om_m2=([128, NMB, HD], BF16),
                rz1=([128, NMB], BF16),
                rz1h=([128, NMB], BF16),
                rzav=([128, NMB], BF16),
                qT=([64, S], BF16),
                kT=([64, S], BF16),
            )
            return {k: hp.tile(sh, dt, tag=f"{k}{s}", name=f"{k}_h{h}")
                    for k, (sh, dt) in shapes.items()}

        def emit_setup_a(h, tl):
            """projections + v/state0 (no exp yet). Engine use: PE/Pool/DVE."""
            X, XF8, qT, kT = tl["X"], tl["XF8"], tl["qT"], tl["kT"]
            om_st = scr.tile([128, HD], F32, tag="om_st")
            src = bass.AP(tensor=om_d.tensor, offset=om_d.offset + h * HD,
                          ap=[[0, 128], [1, HD]])
            nc.sync.dma_start(out=om_st, in_=src)
            om_view = bass.AP(tensor=om_st.tensor, offset=om_st.offset,
                              ap=[om_st.ap[0], [0, NMB], om_st.ap[1]])
            nc.scalar.activation(out=tl["om_p"], in_=om_view,
                                 func=AF.Copy, scale=DT)
            nc.scalar.activation(out=tl["om_m"], in_=om_view,
                                 func=AF.Copy, scale=-DT)
            nc.scalar.activation(out=tl["om_p2"], in_=om_view,
                                 func=AF.Copy, scale=0.5 * DT)
            nc.scalar.activation(out=tl["om_m2"], in_=om_view,
                                 func=AF.Copy, scale=-0.5 * DT)
            xhT = scr.tile([64, S], BF16, tag="xhT")
            for t in range(NMB):
                pt = pmp.tile([128, 512], BF16, tag="pm", name=f"xt{h}_{t}")
                nc.tensor.transpose(pt[0:64, 0:128],
                                    xnh[:, t, h * HD:(h + 1) * HD], identb)
                nc.vector.tensor_copy(out=xhT[:, t * 128:(t + 1) * 128],
                                      in_=pt[0:64, 0:128])
            for sl in range(NSL):
                pq = pmp.tile([128, 512], F32, tag="pm", name=f"pq{h}_{sl}")
                nc.tensor.matmul(pq[0:64, :], wq_sb[:, h, :],
                                 xhT[:, sl * 512:(sl + 1) * 512],
                                 start=True, stop=True)
                nc.scalar.copy(out=qT[:, sl * 512:(sl + 1) * 512],
                               in_=pq[0:64, :])
                pk = pmp.tile([128, 512], F32, tag="pm", name=f"pk{h}_{sl}")
                nc.tensor.matmul(pk[0:64, :], wk_sb[:, h, :],
                                 xhT[:, sl * 512:(sl + 1) * 512],
                                 start=True, stop=True)
                nc.scalar.copy(out=kT[:, sl * 512:(sl + 1) * 512],
                               in_=pk[0:64, :])
            # state0: X = [c1 v | -c2 v], XF8 = [(c1-R21 c2) v | -2 c2 v]
            for t in range(NMB):
                pv = pmp.tile([128, 512], F32, tag="pm", name=f"pv{h}_{t}")
                nc.tensor.matmul(pv[:, 0:HD], xhT[:, t * 128:(t + 1) * 128],
                                 wv_c1[:, h, :], start=True, stop=True)
                nc.tensor.matmul(pv[:, HD:128], xhT[:, t * 128:(t + 1) * 128],
                                 wv_m2[:, h, :], start=True, stop=True)
                nc.tensor.matmul(pv[:, 128:128 + HD],
                                 xhT[:, t * 128:(t + 1) * 128],
                                 wv_r1[:, h, :], start=True, stop=True)
                nc.tensor.matmul(pv[:, 128 + HD:256],
                                 xhT[:, t * 128:(t + 1) * 128],
                                 wv_r2[:, h, :], start=True, stop=True)
                nc.vector.tensor_copy(out=X[:, t, :], in_=pv[:, 0:128])
                nc.vector.tensor_copy(out=XF8[:, t // 2, t % 2, :],
                                      in_=pv[:, 128:256])

        def emit_scores_chunk(h, tl, ck):
            """4 of 16 k-blocks of scores + exp->fp8."""
            qT, kT, ET = tl["qT"], tl["kT"], tl["ET"]
            for kb in range(ck * 4, ck * 4 + 4):
                for sl in range(NSL):
                    ps = pmp.tile([128, 512], F32, tag="pm",
                                  name=f"sc{h}_{kb}_{sl}")
                    nc.tensor.matmul(ps, kT[:, kb * 128:(kb + 1) * 128],
                                     qT[:, sl * 512:(sl + 1) * 512],
                                     start=True, stop=True)
                    nc.scalar.activation(
                        out=ET[:, kb // 2, kb % 2, sl * 512:(sl + 1) * 512],
                        in_=ps, func=AF.Exp, scale=SCL, bias=negtau)

        def emit_setup_b(h, tl):
            """Z row-sums via fp8 ones chain -> rz constants (token-major)."""
            ET = tl["ET"]
            zrow = scr.tile([1, S], F32, tag="zrow")
            for sl in range(NSL):
                psz = pop.tile([128, 512], F32, tag="po", name=f"psz{h}_{sl}")
                for k2 in range(NK2):
                    nc.tensor.matmul(psz[0:16, :], ones8,
                                     ET[:, k2, :, sl * 512:(sl + 1) * 512],
                                     start=(k2 == 0), stop=(k2 == NK2 - 1),
                                     perf_mode=DR)
                nc.scalar.copy(out=zrow[:, sl * 512:(sl + 1) * 512],
                               in_=psz[0:1, :])
            nc.sync.dma_start(out=ztmp[h:h + 1, :], in_=zrow[0:1, :])
            zt16 = scr.tile([16, 128], F32, tag="zt16")
            nc.sync.dma_start(out=zt16,
                              in_=ztmp[h, :].rearrange("(mb p) -> mb p",
                                                       p=128))
            pz = pmp.tile([128, 512], F32, tag="pm", name=f"pz{h}")
            nc.tensor.transpose(pz[:, 0:NMB], zt16, ident[0:16, 0:16])
            ztr = scr.tile([128, NMB], F32, tag="ztr")
            nc.vector.reciprocal(out=ztr, in_=pz[:, 0:NMB])
            nc.vector.tensor_scalar(out=tl["rz1"], in0=ztr, scalar1=DT * C1V,
                                    scalar2=None, op0=ALU.mult)
            nc.vector.tensor_scalar(out=tl["rz1h"], in0=ztr,
                                    scalar1=0.5 * DT * C1V,
                                    scalar2=None, op0=ALU.mult)
            nc.vector.tensor_scalar(out=tl["rzav"], in0=ztr,
                                    scalar1=MIX / (C1V - R21 * C2V),
                                    scalar2=None, op0=ALU.mult)

        def emit_pass(h, p, tl):
            odd = (p % 2 == 1)
            ET, X, PRED, XF8 = tl["ET"], tl["X"], tl["PRED"], tl["XF8"]
            Xi = X if odd else PRED
            Xo = PRED if odd else X
            xf8v = XF8.rearrange("p a b c -> p (a b) c")
            rzc = tl["rz1"] if odd else tl["rz1h"]
            omp = tl["om_p"] if odd else tl["om_p2"]
            omm = tl["om_m"] if odd else tl["om_m2"]
            csc = -DT * INVK2 if odd else -0.5 * DT * INVK2
            cbi = 1.0 + DT * CC1 if odd else 0.5 + 0.5 * DT * CC1
            pos = [None] * NSL
            for half in range(2):
                for s in (2 * half, 2 * half + 1):
                    pos[s] = pop.tile([128, 512], F32, tag="po",
                                      name=f"po{h}_{p}_{s}")
                for k2 in range(NK2):
                    for s in (2 * half, 2 * half + 1):
                        nc.tensor.matmul(pos[s], XF8[:, k2, :, :],
                                         ET[:, k2, :, s * 512:(s + 1) * 512],
                                         start=(k2 == 0),
                                         stop=(k2 == NK2 - 1),
                                         perf_mode=DR)
            for s in range(NSL):
                mb = slice(4 * s, 4 * s + 4)
                ptc = ptp.tile([128, 512], BF16, tag="ptc")
                nc.scalar.copy(out=ptc, in_=pos[s])
                ps2 = ps2p.tile([128, 4, 128], BF16, tag="ps2")
                for t in range(4):
                    nc.tensor.transpose(ps2[:, t, :],
                                        ptc[:, t * 128:(t + 1) * 128], identb)
                RZ = scr.tile([128, 4, 128], BF16, tag=f"RZ{s % 2}")
                nc.vector.tensor_mul(out=RZ, in0=ps2,
                                     in1=bcast(rzc, 4 * s, 4, 128))
                if p == 1:
                    nc.vector.tensor_mul(out=tl["mix"][:, mb, :],
                                         in0=ps2[:, :, 0:HD],
                                         in1=bcast(tl["rzav"], 4 * s, 4, HD))
                XiS = Xi[:, mb, :]
                # x2 morphs: |z|^2 -> growth coefficient -> u = GA*Xi
                x2 = scr.tile([128, 4, 128], BF16, tag=f"x2{s % 2}")
                nc.scalar.activation(out=x2, in_=XiS, func=AF.Square,
                                     scale=1.0)
                nc.vector.tensor_add(out=x2[:, :, 0:HD], in0=x2[:, :, 0:HD],
                                     in1=x2[:, :, HD:128])
                nc.scalar.activation(out=x2[:, :, 0:HD], in_=x2[:, :, 0:HD],
                                     func=AF.Copy, scale=csc, bias=cbi)
                nc.vector.tensor_copy(out=x2[:, :, HD:128],
                                      in_=x2[:, :, 0:HD])
                nc.vector.tensor_mul(out=x2, in0=x2, in1=XiS)
                osw = scr.tile([128, 4, 128], BF16, tag=f"osw{s % 2}")
                nc.vector.tensor_mul(out=osw[:, :, 0:HD],
                                     in0=omm[:, mb, :],
                                     in1=XiS[:, :, HD:128])
                nc.gpsimd.tensor_mul(out=osw[:, :, HD:128],
                                     in0=omp[:, mb, :],
                                     in1=XiS[:, :, 0:HD])
                nc.vector.tensor_add(out=x2, in0=x2, in1=osw)
                if odd:
                    # PRED = GA*X + osw + RZ  (= X + dt f(X))
                    nc.vector.tensor_add(out=PRED[:, mb, :], in0=x2, in1=RZ)
                else:
                    # X_new = 0.5 X + [ (0.5+dt/2 cd)*PRED + osw/2 + RZ/2 ]
                    nc.vector.tensor_add(out=x2, in0=x2, in1=RZ)
                    xh = scr.tile([128, 4, 128], BF16, tag=f"xh{s % 2}")
                    nc.vector.tensor_scalar(out=xh, in0=X[:, mb, :],
                                            scalar1=0.5, scalar2=None,
                                            op0=ALU.mult)
                    nc.vector.tensor_add(out=X[:, mb, :], in0=xh, in1=x2)
                if p < NPASS:
                    nc.vector.scalar_tensor_tensor(
                        out=xf8v[:, mb, 0:HD],
                        in0=Xo[:, mb, HD:128], scalar=R21,
                        in1=Xo[:, mb, 0:HD], op0=ALU.mult, op1=ALU.add)
                    nc.vector.scalar_tensor_tensor(
                        out=xf8v[:, mb, HD:128],
                        in0=Xo[:, mb, 0:HD], scalar=-R21,
                        in1=Xo[:, mb, HD:128], op0=ALU.mult, op1=ALU.add)

        def emit_readout(h, tl):
            X, mix = tl["X"], tl["mix"]
            mo = scr.tile([128, NMB, HD], BF16, tag="mo")
            nc.vector.scalar_tensor_tensor(out=mo, in0=X[:, :, 0:HD],
                                           scalar=M2, in1=mix,
                                           op0=ALU.mult, op1=ALU.add)
            nc.vector.scalar_tensor_tensor(out=mo, in0=X[:, :, HD:128],
                                           scalar=-M3, in1=mo,
                                           op0=ALU.mult, op1=ALU.add)
            for t in range(NMB):
                pt = pmp.tile([128, 512], BF16, tag="pm", name=f"ro{h}_{t}")
                nc.tensor.transpose(pt[0:64, 0:128], mo[:, t, :], identb)
                mt = scr.tile([64, 128], BF16, tag="mt")
                nc.scalar.copy(out=mt, in_=pt[0:64, 0:128])
                po = pmp.tile([128, 512], F32, tag="pm", name=f"rp{h}_{t}")
                nc.tensor.matmul(po[:, 0:HD], mt, wo_sb[:, h, :],
                                 start=True, stop=True)
                nc.scalar.copy(out=xattn[:, t, h * HD:(h + 1) * HD],
                               in_=po[:, 0:HD])

        # --------- pipelined schedule: setup(h+1) hides under passes(h) ----
        tls = {}
        tls[0] = head_tiles(0)
        emit_setup_a(0, tls[0])
        for ck in range(4):
            emit_scores_chunk(0, tls[0], ck)
        emit_setup_b(0, tls[0])
        for h in range(NHL):
            if h + 1 < NHL:
                tls[h + 1] = head_tiles(h + 1)
                emit_setup_a(h + 1, tls[h + 1])
            for p in range(1, NPASS + 1):
                emit_pass(h, p, tls[h])
                if h + 1 < NHL and p <= 4:
                    emit_scores_chunk(h + 1, tls[h + 1], p - 1)
                if h + 1 < NHL and p == 6:
                    emit_setup_b(h + 1, tls[h + 1])
            emit_readout(h, tls[h])
            del tls[h]

    # ======================= AllToAll =======================
    with tc.tile_pool(name="ccs", bufs=8) as ccs:
        for j in range(N_CORES):
            for tt in range(TT4):
                mb0 = (j % 4) * TT4 + tt
                stg = ccs.tile([128, HC], F32, tag="stg")
                nc.vector.tensor_scalar_mul(out=stg, in0=xattn[:, mb0, :],
                                            scalar1=maskbc[:, j:j + 1])
                nc.sync.dma_start(
                    out=cc_in[j * TOK + tt * 128:j * TOK + (tt + 1) * 128, :],
                    in_=stg)
        if fake_cc:
            nc.sync.dma_start(out=cc_out, in_=cc_in)
        else:
            nc.gpsimd.collective_compute(
                "AllToAll", ALU.bypass,
                replica_groups=[list(range(N_CORES))],
                ins=[cc_in.opt()], outs=[cc_out.opt()])

    # ======================= FFN =======================
    with tc.tile_pool(name="ffw", bufs=1) as ffw, \
         tc.tile_pool(name="ffa", bufs=3) as ffa, \
         tc.tile_pool(name="ffs", bufs=4) as ffs, \
         tc.tile_pool(name="w1p", bufs=4) as w1p, \
         tc.tile_pool(name="w2p", bufs=3) as w2p, \
         tc.tile_pool(name="psf", bufs=2, space="PSUM") as psfp, \
         tc.tile_pool(name="pso", bufs=1, space="PSUM") as psop, \
         tc.tile_pool(name="pstf", bufs=2, space="PSUM") as pstf:

        g2bc = ffw.tile([128, D], F32)
        nc.sync.dma_start(out=g2bc, in_=g2_d[None, :].to_broadcast([128, D]))
        be2bc = ffw.tile([128, D], F32)
        nc.sync.dma_start(out=be2bc, in_=be2_d[None, :].to_broadcast([128, D]))
        bf2bc = ffw.tile([128, D], F32)
        nc.sync.dma_start(out=bf2bc, in_=bf2_d[None, :].to_broadcast([128, D]))
        bf1sb = ffw.tile([128, DFF // 128], F32)
        nc.sync.dma_start(out=bf1sb, in_=bf1_d.rearrange("(f p) -> p f", p=128))
        bf1h = ffw.tile([128, DFF // 128], F32)
        nc.scalar.activation(out=bf1h, in_=bf1sb, func=AF.Copy, scale=0.5)
        x1_all = ffw.tile([128, TT4, D], F32)
        xn1T = ffw.tile([128, D // 128, TOK], F32R)
        hT = ffw.tile([128, DFF // 128, TOK], BF16)

        cc_a = ffw.tile([128, TT4, D], F32)
        cc_b = ffw.tile([128, TT4, D], F32)
        for tt in range(TT4):
            for kk in range(4):
                nc.sync.dma_start(out=cc_a[:, tt, kk * HC:(kk + 1) * HC],
                                  in_=cc_out[kk * TOK + tt * 128:
                                             kk * TOK + (tt + 1) * 128, :])
                nc.sync.dma_start(out=cc_b[:, tt, kk * HC:(kk + 1) * HC],
                                  in_=cc_out[(kk + 4) * TOK + tt * 128:
                                             (kk + 4) * TOK + (tt + 1) * 128, :])
        for tt in range(TT4):
            xa = ffa.tile([128, D], F32, tag="xa")
            nc.vector.tensor_add(out=xa, in0=cc_a[:, tt, :], in1=cc_b[:, tt, :])
            xtk = ffa.tile([128, D], F32, tag="xtk")
            nc.sync.dma_start(out=xtk, in_=x_tok[tt * 128:(tt + 1) * 128, :])
            nc.gpsimd.tensor_add(out=x1_all[:, tt, :], in0=xtk, in1=xa)
            # LN2
            st = ffs.tile([128, 2, 6], F32, tag="st")
            for sg in range(2):
                nc.vector.bn_stats(out=st[:, sg, :],
                                   in_=x1_all[:, tt, sg * 512:(sg + 1) * 512])
            mv = ffs.tile([128, 2], F32, tag="mv")
            nc.vector.bn_aggr(out=mv, in_=st)
            rstd = ffs.tile([128, 1], F32, tag="rstd")
            nc.scalar.activation(out=rstd, in_=mv[:, 1:2], func=AF.Sqrt,
                                 bias=epsT, scale=1.0)
            nc.vector.reciprocal(out=rstd, in_=rstd)
            xn1 = ffa.tile([128, D], F32, tag="xn1")
            nc.vector.tensor_scalar(out=xn1, in0=x1_all[:, tt, :],
                                    scalar1=mv[:, 0:1], scalar2=rstd,
                                    op0=ALU.subtract, op1=ALU.mult)
            nc.vector.tensor_mul(out=xn1, in0=xn1, in1=g2bc)
            nc.gpsimd.tensor_add(out=xn1, in0=xn1, in1=be2bc)
            for dd in range(D // 128):
                pt = pstf.tile([128, 128], F32, tag="pt")
                nc.tensor.transpose(pt, xn1[:, dd * 128:(dd + 1) * 128], ident)
                nc.scalar.copy(out=xn1T[:, dd, tt * 128:(tt + 1) * 128], in_=pt)

        # h^T = gelu(W1^T @ xn1^T + bf1)
        for f in range(DFF // 128):
            w1f = w1p.tile([128, D // 128, 128], F32, tag="w1f")
            nc.sync.dma_start(
                out=w1f,
                in_=w1_d.rearrange("(dd p) ff -> p dd ff",
                                   p=128)[:, :, f * 128:(f + 1) * 128])
            w1fr = w1p.tile([128, D // 128, 128], F32R, tag="w1fr")
            nc.gpsimd.tensor_copy(out=w1fr, in_=w1f)
            ph = psfp.tile([128, TOK], F32, tag="ph")
            for dd in range(D // 128):
                nc.tensor.matmul(ph, w1fr[:, dd, :], xn1T[:, dd, :],
                                 start=(dd == 0), stop=(dd == D // 128 - 1))
            # gelu (tanh approx) on y = x/2:
            gy = ffa.tile([128, TOK], F32, tag="gy")
            nc.scalar.activation(out=gy, in_=ph, func=AF.Identity, scale=0.5,
                                 bias=bf1h[:, f:f + 1])
            gt = ffa.tile([128, TOK], F32, tag="gt")
            nc.scalar.activation(out=gt, in_=gy, func=AF.Square, scale=1.0)
            nc.vector.tensor_scalar(out=gt, in0=gt, scalar1=8 * 0.044715 * GC0,
                                    scalar2=2 * GC0, op0=ALU.mult, op1=ALU.add)
            nc.vector.tensor_mul(out=gt, in0=gt, in1=gy)
            nc.scalar.activation(out=gt, in_=gt, func=AF.Tanh, scale=1.0)
            nc.vector.scalar_tensor_tensor(out=hT[:, f, :], in0=gt, scalar=1.0,
                                           in1=gy, op0=ALU.add, op1=ALU.mult)

        # out = x1 + h @ W2 + bf2   (W2 streamed, bf16)
        for dh in range(D // 512):
            pos = [psop.tile([128, 512], F32, tag=f"po{tt}", name=f"po{tt}")
                   for tt in range(TT4)]
            for f in range(DFF // 128):
                w2s = w2p.tile([128, 512], F32, tag="w2s")
                nc.sync.dma_start(out=w2s,
                                  in_=w2_d[f * 128:(f + 1) * 128,
                                           dh * 512:(dh + 1) * 512])
                w2b = w2p.tile([128, 512], BF16, tag="w2b")
                nc.gpsimd.tensor_copy(out=w2b, in_=w2s)
                for tt in range(TT4):
                    nc.tensor.matmul(pos[tt], hT[:, f, tt * 128:(tt + 1) * 128],
                                     w2b, start=(f == 0),
                                     stop=(f == DFF // 128 - 1))
            for tt in range(TT4):
                o1 = ffa.tile([128, 512], F32, tag="o1")
                nc.vector.tensor_add(out=o1, in0=pos[tt],
                                     in1=x1_all[:, tt, dh * 512:(dh + 1) * 512])
                nc.vector.tensor_add(out=o1, in0=o1,
                                     in1=bf2bc[:, dh * 512:(dh + 1) * 512])
                nc.sync.dma_start(out=out_d[tt * 128:(tt + 1) * 128,
                                            dh * 512:(dh + 1) * 512], in_=o1)

    ctx.close()


# ======================= host-side driver =======================

def shard_inputs(inputs, S=S_FULL):
    x = np.ascontiguousarray(inputs["x"], dtype=np.float32)
    TOK = S // 4
    in_maps = []
    for c in range(N_CORES):
        b = c // 4
        hg = c % 4
        hsl = slice(hg * NHL, (hg + 1) * NHL)
        csl = slice(hg * NHL * HD, (hg + 1) * NHL * HD)
        rsl = slice(hg * TOK, (hg + 1) * TOK)
        m = {
            "x_full": x[b],
            "x_heads": x[b][:, csl],
            "x_tok": x[b][rsl, :],
            "wq": inputs["Wq"][hsl].reshape(NHL * HD, HD),
            "wk": inputs["Wk"][hsl].reshape(NHL * HD, HD),
            "wv": inputs["Wv"][hsl].reshape(NHL * HD, HD),
            "wo": inputs["Wo"][hsl].reshape(NHL * HD, HD),
            "omega": inputs["omega"][hsl],
            "g1h": inputs["g1"][csl],
            "be1h": inputs["be1"][csl],
            "g2": inputs["g2"], "be2": inputs["be2"],
            "w1": inputs["W1"], "bf1": inputs["bf1"],
            "w2": inputs["W2"], "bf2": inputs["bf2"],
            "gmask": np.array([1.0 if j // 4 == b else 0.0
                               for j in range(N_CORES)], dtype=np.float32),
        }
        in_maps.append({k: np.ascontiguousarray(v, dtype=np.float32)
                        for k, v in m.items()})
    return in_maps


def assemble_output(results, S=S_FULL):
    TOK = S // 4
    out = np.zeros((B, S, D), dtype=np.float32)
    for c in range(N_CORES):
        b, hg = c // 4, c % 4
        out[b, hg * TOK:(hg + 1) * TOK, :] = results[c]["out"]
    return out


_NC_CACHE = {}


def kernel(**inputs):
    from concourse.bass_utils import run_bass_kernel_spmd
    S = inputs["x"].shape[1]
    if S not in _NC_CACHE:
        _NC_CACHE[S] = build_nc(S)
    nc = _NC_CACHE[S]
    in_maps = shard_inputs(inputs, S)
    res = run_bass_kernel_spmd(nc, in_maps, core_ids=list(range(N_CORES)))
    return assemble_output(res.results, S)


# revision 27
# speedup vs baseline: 1.0159x; 1.0159x over previous
"""Trainium2 Bass kernel for MinimalResonanceLayer (8-core SPMD).

Sharding: core c handles batch b = c//4 and local heads [ (c%4)*4, (c%4)*4+4 ).
Head-concat + FFN via one 8-core AllToAll (per-core divergence encoded in an
input mask so the program stays SPMD-uniform).

Fast path vs v1:
  - Heun integrator with 4 steps (dt=0.025) instead of 5 (error ~3.5e-3,
    well under the 2e-2 gate).
  - E (exp of scores) stored in fp8-e4m3 with a constant exp-shift tau;
    A@state matmuls run in DoubleRow fp8 mode, out^T-form (stationary =
    state blocks [k,2j,128], moving = E^T slices [k,2j,512]) -> 4x fewer,
    4x cheaper PE matmuls than the N=128 bf16 form.
  - Row-sums Z from an fp8 ones-stationary chain; 1/Z is carried to
    token-major via a DRAM round-trip and folded into per-pass constants.
  - Elementwise update in token-major (PE transposes of the psum), with
    dt folded into all constants and ops spread across DVE/Pool/Act.
"""
import math
import numpy as np

import concourse.bass as bass
import concourse.tile as tile
from concourse import bacc, mybir
from concourse.masks import make_identity

# ---- problem constants (hardcoded per contest contract) ----
B, S_FULL, D, H, HD = 2, 2048, 1024, 16, 64
DFF = 2 * D
MU, ALPHA, K_COUP, MIX = 1.0, 0.1, 3.0, 0.3
T_TOTAL = 0.1            # 5 ref steps x dt 0.02
NSTEPS = 4               # our Heun steps
DT = T_TOTAL / NSTEPS    # 0.025
NPASS = 2 * NSTEPS       # 8 f-evals
N_CORES = 8
NHL = 4                  # heads per core

CA, SA = math.cos(ALPHA), math.sin(ALPHA)
C1V = K_COUP * CA            # K cos(alpha)
C2V = K_COUP * SA
CC1 = MU - K_COUP            # -2.0
R21 = C2V / C1V              # tan(alpha)
M2 = (1.0 - MIX) * CA / K_COUP
M3 = (1.0 - MIX) * SA / K_COUP
SCL = 1.0 / math.sqrt(HD)
INVK2 = 1.0 / (K_COUP * K_COUP)
TAU = 2.5                    # constant exp shift keeping E in fp8-e4m3 range
GC0 = math.sqrt(2.0 / math.pi)

F32 = mybir.dt.float32
F32R = mybir.dt.float32r
BF16 = mybir.dt.bfloat16
F8 = mybir.dt.float8e4
ALU = mybir.AluOpType
AF = mybir.ActivationFunctionType
DR = mybir.MatmulPerfMode.DoubleRow


def bcast(t2d, mb0, nmb, w):
    """[128, nmb, w] stride-0 free-broadcast view of t2d[:, mb0:mb0+nmb]."""
    p0, f0 = t2d.ap[0], t2d.ap[1]
    return bass.AP(tensor=t2d.tensor, offset=t2d.offset + mb0 * f0[0],
                   ap=[p0, [f0[0], nmb], [0, w]])


def build_nc(S=S_FULL, fake_cc=False):
    nc = bacc.Bacc("TRN2", target_bir_lowering=False, debug=False,
                   num_devices=N_CORES)

    def din(name, shape):
        return nc.dram_tensor(name, shape, F32, kind="ExternalInput").ap()

    TOK = S // 4
    io = dict(
        x_full=din("x_full", [S, D]),
        x_heads=din("x_heads", [S, NHL * HD]),
        x_tok=din("x_tok", [TOK, D]),
        wq_d=din("wq", [NHL * HD, HD]),
        wk_d=din("wk", [NHL * HD, HD]),
        wv_d=din("wv", [NHL * HD, HD]),
        wo_d=din("wo", [NHL * HD, HD]),
        om_d=din("omega", [NHL, HD]),
        g1_d=din("g1h", [NHL * HD]),
        be1_d=din("be1h", [NHL * HD]),
        g2_d=din("g2", [D]),
        be2_d=din("be2", [D]),
        w1_d=din("w1", [D, DFF]),
        bf1_d=din("bf1", [DFF]),
        w2_d=din("w2", [DFF, D]),
        bf2_d=din("bf2", [D]),
        gm_d=din("gmask", [N_CORES]),
        out_d=nc.dram_tensor("out", [TOK, D], F32, kind="ExternalOutput").ap(),
    )

    with tile.TileContext(nc) as tc:
        _body(nc, tc, io, S, fake_cc)

    nc.compile()
    return nc


def _body(nc, tc, io, S, fake_cc=False):
    NMB = S // 128          # 16 token blocks
    NK2 = NMB // 2          # 8 double-row k-groups
    NSL = S // 512          # 4 column slices
    TOK = S // 4
    TT4 = TOK // 128
    HC = NHL * HD

    x_full, x_heads, x_tok = io["x_full"], io["x_heads"], io["x_tok"]
    wq_d, wk_d, wv_d, wo_d = io["wq_d"], io["wk_d"], io["wv_d"], io["wo_d"]
    om_d, g1_d, be1_d = io["om_d"], io["g1_d"], io["be1_d"]
    g2_d, be2_d = io["g2_d"], io["be2_d"]
    w1_d, bf1_d, w2_d, bf2_d = io["w1_d"], io["bf1_d"], io["w2_d"], io["bf2_d"]
    gm_d, out_d = io["gm_d"], io["out_d"]

    from contextlib import ExitStack
    ctx = ExitStack()
    sing = ctx.enter_context(tc.tile_pool(name="sing", bufs=1))
    dram = ctx.enter_context(tc.tile_pool(name="dram", bufs=1, space="DRAM"))

    # ---- whole-kernel constants ----
    ident = sing.tile([128, 128], F32)
    make_identity(nc, ident)
    identb = sing.tile([128, 128], BF16)
    nc.vector.tensor_copy(out=identb, in_=ident)
    epsT = sing.tile([128, 1], F32)
    nc.vector.memset(epsT, 1e-5)
    negtau = sing.tile([128, 1], F32)
    nc.vector.memset(negtau, -TAU)
    ones8 = sing.tile([128, 2, 16], F8)
    onesf = sing.tile([128, 2, 16], F32)
    nc.vector.memset(onesf, 1.0)
    nc.vector.tensor_copy(out=ones8, in_=onesf)
    maskbc = sing.tile([128, N_CORES], F32)
    nc.sync.dma_start(out=maskbc, in_=gm_d[None, :].to_broadcast([128, N_CORES]))
    xattn = sing.tile([128, NMB, HC], BF16)

    cc_in = dram.tile([N_CORES * TOK, HC], F32)
    cc_out = dram.tile([N_CORES * TOK, HC], F32)
    ztmp = dram.tile([NHL, S], F32)

    # =================== attention super-phase ===================
    with ExitStack() as actx:
        big = actx.enter_context(tc.tile_pool(name="big", bufs=1))
        g1bc = big.tile([128, HC], F32)
        nc.sync.dma_start(out=g1bc, in_=g1_d[None, :].to_broadcast([128, HC]))
        be1bc = big.tile([128, HC], F32)
        nc.sync.dma_start(out=be1bc, in_=be1_d[None, :].to_broadcast([128, HC]))
        wstage = big.tile([64, 4, NHL, HD], F32)
        for wi, wd in enumerate([wq_d, wk_d, wv_d, wo_d]):
            nc.sync.dma_start(out=wstage[:, wi, :, :],
                              in_=wd.rearrange("(h p) e -> p h e", p=HD))
        wq_sb = big.tile([64, NHL, HD], BF16)
        nc.vector.tensor_copy(out=wq_sb, in_=wstage[:, 0, :, :])
        wk_sb = big.tile([64, NHL, HD], BF16)
        nc.vector.tensor_copy(out=wk_sb, in_=wstage[:, 1, :, :])
        wv_sb = big.tile([64, NHL, HD], BF16)
        nc.vector.tensor_copy(out=wv_sb, in_=wstage[:, 2, :, :])
        wo_sb = big.tile([64, NHL, HD], BF16)
        nc.vector.tensor_copy(out=wo_sb, in_=wstage[:, 3, :, :])

        xnh = big.tile([128, NMB, HC], BF16)

        # ---------------- LN1 ----------------
        with tc.tile_pool(name="ln", bufs=4) as ln, \
             tc.tile_pool(name="lns", bufs=6) as lns, \
             tc.tile_pool(name="lnp", bufs=2, space="PSUM") as lnp:
            for t in range(NMB):
                xt = ln.tile([128, D], F32, tag="xt")
                nc.sync.dma_start(out=xt, in_=x_full[t * 128:(t + 1) * 128, :])
                xh = ln.tile([128, HC], F32, tag="xh")
                nc.sync.dma_start(out=xh, in_=x_heads[t * 128:(t + 1) * 128, :])
                st = lns.tile([128, 2, 6], F32, tag="st")
                for sg in range(2):
                    nc.vector.bn_stats(out=st[:, sg, :],
                                       in_=xt[:, sg * 512:(sg + 1) * 512])
                mv = lns.tile([128, 2], F32, tag="mv")
                nc.vector.bn_aggr(out=mv, in_=st)
                rstd = lns.tile([128, 1], F32, tag="rstd")
                nc.scalar.activation(out=rstd, in_=mv[:, 1:2], func=AF.Sqrt,
                                     bias=epsT, scale=1.0)
                nc.vector.reciprocal(out=rstd, in_=rstd)
                nb = lns.tile([128, 1], F32, tag="nb")
                nc.vector.tensor_scalar(out=nb, in0=mv[:, 0:1], scalar1=rstd,
                                        scalar2=-1.0, op0=ALU.mult, op1=ALU.mult)
                xs = lns.tile([128, HC], F32, tag="xs")
                nc.scalar.activation(out=xs, in_=xh, func=AF.Identity,
                                     scale=rstd, bias=nb)
                nc.vector.tensor_mul(out=xs, in0=xs, in1=g1bc)
                nc.gpsimd.tensor_add(out=xnh[:, t, :], in0=xs, in1=be1bc)

        # ---------------- per-head pipelined resonance ----------------
        hp = actx.enter_context(tc.tile_pool(name="hp", bufs=1))
        scr = actx.enter_context(tc.tile_pool(name="scr", bufs=1))
        ptp = actx.enter_context(tc.tile_pool(name="ptp", bufs=2))
        pop = actx.enter_context(tc.tile_pool(name="pop", bufs=5, space="PSUM"))
        ps2p = actx.enter_context(tc.tile_pool(name="ps2p", bufs=1, space="PSUM"))
        pmp = actx.enter_context(tc.tile_pool(name="pmp", bufs=2, space="PSUM"))

        # wv variants with state-build scales folded in
        wv_c1 = big.tile([64, NHL, HD], BF16)
        nc.vector.tensor_scalar(out=wv_c1, in0=wstage[:, 2, :, :],
                                scalar1=C1V, scalar2=None, op0=ALU.mult)
        wv_m2 = big.tile([64, NHL, HD], BF16)
        nc.vector.tensor_scalar(out=wv_m2, in0=wstage[:, 2, :, :],
                                scalar1=-C2V, scalar2=None, op0=ALU.mult)
        wv_r1 = big.tile([64, NHL, HD], BF16)
        nc.vector.tensor_scalar(out=wv_r1, in0=wstage[:, 2, :, :],
                                scalar1=C1V - R21 * C2V, scalar2=None,
                                op0=ALU.mult)
        wv_r2 = big.tile([64, NHL, HD], BF16)
        nc.vector.tensor_scalar(out=wv_r2, in0=wstage[:, 2, :, :],
                                scalar1=-2.0 * C2V, scalar2=None,
                                op0=ALU.mult)

        def head_tiles(h):
            s = h % 2
            shapes = dict(
                ET=([128, NK2, 2, S], F8),
                X=([128, NMB, 128], BF16),
                PRED=([128, NMB, 128], BF16),
                XF8=([128, NK2, 2, 128], F8),
                mix=([128, NMB, HD], BF16),
                om_p=([128, NMB, HD], BF16),
                om_m=([128, NMB, HD], BF16),
                om_p2=([128, NMB, HD], BF16),
                om_m2=([128, NMB, HD], BF16),
                rz1=([128, NMB], BF16),
                rz1h=([128, NMB], BF16),
                rzav=([128, NMB], BF16),
                qT=([64, S], BF16),
                kT=([64, S], BF16),
            )
            return {k: hp.tile(sh, dt, tag=f"{k}{s}", name=f"{k}_h{h}")
                    for k, (sh, dt) in shapes.items()}

        def emit_setup_a(h, tl):
            """projections + v/state0 (no exp yet). Engine use: PE/Pool/DVE."""
            X, XF8, qT, kT = tl["X"], tl["XF8"], tl["qT"], tl["kT"]
            om_st = scr.tile([128, HD], F32, tag="om_st")
            src = bass.AP(tensor=om_d.tensor, offset=om_d.offset + h * HD,
                          ap=[[0, 128], [1, HD]])
            nc.sync.dma_start(out=om_st, in_=src)
            om_view = bass.AP(tensor=om_st.tensor, offset=om_st.offset,
                              ap=[om_st.ap[0], [0, NMB], om_st.ap[1]])
            nc.scalar.activation(out=tl["om_p"], in_=om_view,
                                 func=AF.Copy, scale=DT)
            nc.scalar.activation(out=tl["om_m"], in_=om_view,
                                 func=AF.Copy, scale=-DT)
            nc.scalar.activation(out=tl["om_p2"], in_=om_view,
                                 func=AF.Copy, scale=0.5 * DT)
            nc.scalar.activation(out=tl["om_m2"], in_=om_view,
                                 func=AF.Copy, scale=-0.5 * DT)
            xhT = scr.tile([64, S], BF16, tag="xhT")
            for t in range(NMB):
                pt = pmp.tile([128, 512], BF16, tag="pm", name=f"xt{h}_{t}")
                nc.tensor.transpose(pt[0:64, 0:128],
                                    xnh[:, t, h * HD:(h + 1) * HD], identb)
                nc.vector.tensor_copy(out=xhT[:, t * 128:(t + 1) * 128],
                                      in_=pt[0:64, 0:128])
            for sl in range(NSL):
                pq = pmp.tile([128, 512], F32, tag="pm", name=f"pq{h}_{sl}")
                nc.tensor.matmul(pq[0:64, :], wq_sb[:, h, :],
                                 xhT[:, sl * 512:(sl + 1) * 512],
                                 start=True, stop=True)
                nc.scalar.copy(out=qT[:, sl * 512:(sl + 1) * 512],
                               in_=pq[0:64, :])
                pk = pmp.tile([128, 512], F32, tag="pm", name=f"pk{h}_{sl}")
                nc.tensor.matmul(pk[0:64, :], wk_sb[:, h, :],
                                 xhT[:, sl * 512:(sl + 1) * 512],
                                 start=True, stop=True)
                nc.scalar.copy(out=kT[:, sl * 512:(sl + 1) * 512],
                               in_=pk[0:64, :])
            # state0: X = [c1 v | -c2 v], XF8 = [(c1-R21 c2) v | -2 c2 v]
            for t in range(NMB):
                pv = pmp.tile([128, 512], F32, tag="pm", name=f"pv{h}_{t}")
                nc.tensor.matmul(pv[:, 0:HD], xhT[:, t * 128:(t + 1) * 128],
                                 wv_c1[:, h, :], start=True, stop=True)
                nc.tensor.matmul(pv[:, HD:128], xhT[:, t * 128:(t + 1) * 128],
                                 wv_m2[:, h, :], start=True, stop=True)
                nc.tensor.matmul(pv[:, 128:128 + HD],
                                 xhT[:, t * 128:(t + 1) * 128],
                                 wv_r1[:, h, :], start=True, stop=True)
                nc.tensor.matmul(pv[:, 128 + HD:256],
                                 xhT[:, t * 128:(t + 1) * 128],
                                 wv_r2[:, h, :], start=True, stop=True)
                nc.vector.tensor_copy(out=X[:, t, :], in_=pv[:, 0:128])
                nc.vector.tensor_copy(out=XF8[:, t // 2, t % 2, :],
                                      in_=pv[:, 128:256])

        def emit_scores_chunk(h, tl, ck):
            """4 of 16 k-blocks of scores + exp->fp8."""
            qT, kT, ET = tl["qT"], tl["kT"], tl["ET"]
            for kb in range(ck * 4, ck * 4 + 4):
                for sl in range(NSL):
                    ps = pmp.tile([128, 512], F32, tag="pm",
                                  name=f"sc{h}_{kb}_{sl}")
                    nc.tensor.matmul(ps, kT[:, kb * 128:(kb + 1) * 128],
                                     qT[:, sl * 512:(sl + 1) * 512],
                                     start=True, stop=True)
                    nc.scalar.activation(
                        out=ET[:, kb // 2, kb % 2, sl * 512:(sl + 1) * 512],
                        in_=ps, func=AF.Exp, scale=SCL, bias=negtau)

        def emit_setup_b(h, tl):
            """Z row-sums via fp8 ones chain -> rz constants (token-major)."""
            ET = tl["ET"]
            zrow = scr.tile([1, S], F32, tag="zrow")
            for sl in range(NSL):
                psz = pop.tile([128, 512], F32, tag="po", name=f"psz{h}_{sl}")
                for k2 in range(NK2):
                    nc.tensor.matmul(psz[0:16, :], ones8,
                                     ET[:, k2, :, sl * 512:(sl + 1) * 512],
                                     start=(k2 == 0), stop=(k2 == NK2 - 1),
                                     perf_mode=DR)
                nc.scalar.copy(out=zrow[:, sl * 512:(sl + 1) * 512],
                               in_=psz[0:1, :])
            nc.sync.dma_start(out=ztmp[h:h + 1, :], in_=zrow[0:1, :])
            zt16 = scr.tile([16, 128], F32, tag="zt16")
            nc.sync.dma_start(out=zt16,
                              in_=ztmp[h, :].rearrange("(mb p) -> mb p",
                                                       p=128))
            pz = pmp.tile([128, 512], F32, tag="pm", name=f"pz{h}")
            nc.tensor.transpose(pz[:, 0:NMB], zt16, ident[0:16, 0:16])
            ztr = scr.tile([128, NMB], F32, tag="ztr")
            nc.vector.reciprocal(out=ztr, in_=pz[:, 0:NMB])
            nc.vector.tensor_scalar(out=tl["rz1"], in0=ztr, scalar1=DT * C1V,
                                    scalar2=None, op0=ALU.mult)
            nc.vector.tensor_scalar(out=tl["rz1h"], in0=ztr,
                                    scalar1=0.5 * DT * C1V,
                                    scalar2=None, op0=ALU.mult)
            nc.vector.tensor_scalar(out=tl["rzav"], in0=ztr,
                                    scalar1=MIX / (C1V - R21 * C2V),
                                    scalar2=None, op0=ALU.mult)

        def emit_pass(h, p, tl, act_hot=False):
            odd = (p % 2 == 1)
            ET, X, PRED, XF8 = tl["ET"], tl["X"], tl["PRED"], tl["XF8"]
            Xi = X if odd else PRED
            Xo = PRED if odd else X
            xf8v = XF8.rearrange("p a b c -> p (a b) c")
            rzc = tl["rz1"] if odd else tl["rz1h"]
            omp = tl["om_p"] if odd else tl["om_p2"]
            omm = tl["om_m"] if odd else tl["om_m2"]
            csc = -DT * INVK2 if odd else -0.5 * DT * INVK2
            cbi = 1.0 + DT * CC1 if odd else 0.5 + 0.5 * DT * CC1
            pos = [None] * NSL
            for half in range(2):
                for s in (2 * half, 2 * half + 1):
                    pos[s] = pop.tile([128, 512], F32, tag="po",
                                      name=f"po{h}_{p}_{s}")
                for k2 in range(NK2):
                    for s in (2 * half, 2 * half + 1):
                        nc.tensor.matmul(pos[s], XF8[:, k2, :, :],
                                         ET[:, k2, :, s * 512:(s + 1) * 512],
                                         start=(k2 == 0),
                                         stop=(k2 == NK2 - 1),
                                         perf_mode=DR)
            for s in range(NSL):
                mb = slice(4 * s, 4 * s + 4)
                ptc = ptp.tile([128, 512], BF16, tag="ptc")
                nc.scalar.copy(out=ptc, in_=pos[s])
                ps2 = ps2p.tile([128, 4, 128], BF16, tag="ps2")
                for t in range(4):
                    nc.tensor.transpose(ps2[:, t, :],
                                        ptc[:, t * 128:(t + 1) * 128], identb)
                RZ = scr.tile([128, 4, 128], BF16, tag=f"RZ{s % 2}")
                nc.vector.tensor_mul(out=RZ, in0=ps2,
                                     in1=bcast(rzc, 4 * s, 4, 128))
                if p == 1:
                    nc.vector.tensor_mul(out=tl["mix"][:, mb, :],
                                         in0=ps2[:, :, 0:HD],
                                         in1=bcast(tl["rzav"], 4 * s, 4, HD))
                XiS = Xi[:, mb, :]
                # x2 morphs: |z|^2 -> growth coefficient -> u = GA*Xi
                x2 = scr.tile([128, 4, 128], BF16, tag=f"x2{s % 2}")
                if act_hot:
                    nc.vector.tensor_mul(out=x2, in0=XiS, in1=XiS)
                else:
                    nc.scalar.activation(out=x2, in_=XiS, func=AF.Square,
                                         scale=1.0)
                nc.vector.tensor_add(out=x2[:, :, 0:HD], in0=x2[:, :, 0:HD],
                                     in1=x2[:, :, HD:128])
                nc.scalar.activation(out=x2[:, :, 0:HD], in_=x2[:, :, 0:HD],
                                     func=AF.Copy, scale=csc, bias=cbi)
                nc.vector.tensor_copy(out=x2[:, :, HD:128],
                                      in_=x2[:, :, 0:HD])
                nc.vector.tensor_mul(out=x2, in0=x2, in1=XiS)
                osw = scr.tile([128, 4, 128], BF16, tag=f"osw{s % 2}")
                nc.vector.tensor_mul(out=osw[:, :, 0:HD],
                                     in0=omm[:, mb, :],
                                     in1=XiS[:, :, HD:128])
                nc.gpsimd.tensor_mul(out=osw[:, :, HD:128],
                                     in0=omp[:, mb, :],
                                     in1=XiS[:, :, 0:HD])
                nc.vector.tensor_add(out=x2, in0=x2, in1=osw)
                if odd:
                    # PRED = GA*X + osw + RZ  (= X + dt f(X))
                    nc.vector.tensor_add(out=PRED[:, mb, :], in0=x2, in1=RZ)
                else:
                    # X_new = 0.5 X + [ (0.5+dt/2 cd)*PRED + osw/2 + RZ/2 ]
                    nc.vector.tensor_add(out=x2, in0=x2, in1=RZ)
                    xh = scr.tile([128, 4, 128], BF16, tag=f"xh{s % 2}")
                    nc.vector.tensor_scalar(out=xh, in0=X[:, mb, :],
                                            scalar1=0.5, scalar2=None,
                                            op0=ALU.mult)
                    nc.vector.tensor_add(out=X[:, mb, :], in0=xh, in1=x2)
                if p < NPASS:
                    nc.vector.scalar_tensor_tensor(
                        out=xf8v[:, mb, 0:HD],
                        in0=Xo[:, mb, HD:128], scalar=R21,
                        in1=Xo[:, mb, 0:HD], op0=ALU.mult, op1=ALU.add)
                    nc.vector.scalar_tensor_tensor(
                        out=xf8v[:, mb, HD:128],
                        in0=Xo[:, mb, 0:HD], scalar=-R21,
                        in1=Xo[:, mb, HD:128], op0=ALU.mult, op1=ALU.add)

        def emit_readout(h, tl):
            X, mix = tl["X"], tl["mix"]
            mo = scr.tile([128, NMB, HD], BF16, tag="mo")
            nc.vector.scalar_tensor_tensor(out=mo, in0=X[:, :, 0:HD],
                                           scalar=M2, in1=mix,
                                           op0=ALU.mult, op1=ALU.add)
            nc.vector.scalar_tensor_tensor(out=mo, in0=X[:, :, HD:128],
                                           scalar=-M3, in1=mo,
                                           op0=ALU.mult, op1=ALU.add)
            for t in range(NMB):
                pt = pmp.tile([128, 512], BF16, tag="pm", name=f"ro{h}_{t}")
                nc.tensor.transpose(pt[0:64, 0:128], mo[:, t, :], identb)
                mt = scr.tile([64, 128], BF16, tag="mt")
                nc.scalar.copy(out=mt, in_=pt[0:64, 0:128])
                po = pmp.tile([128, 512], F32, tag="pm", name=f"rp{h}_{t}")
                nc.tensor.matmul(po[:, 0:HD], mt, wo_sb[:, h, :],
                                 start=True, stop=True)
                nc.scalar.copy(out=xattn[:, t, h * HD:(h + 1) * HD],
                               in_=po[:, 0:HD])

        # --------- pipelined schedule: setup(h+1) hides under passes(h) ----
        tls = {}
        tls[0] = head_tiles(0)
        emit_setup_a(0, tls[0])
        for ck in range(4):
            emit_scores_chunk(0, tls[0], ck)
        emit_setup_b(0, tls[0])
        for h in range(NHL):
            if h + 1 < NHL:
                tls[h + 1] = head_tiles(h + 1)
                emit_setup_a(h + 1, tls[h + 1])
            for p in range(1, NPASS + 1):
                emit_pass(h, p, tls[h],
                          act_hot=(h + 1 < NHL and p <= 4))
                if h + 1 < NHL and p <= 4:
                    emit_scores_chunk(h + 1, tls[h + 1], p - 1)
                if h + 1 < NHL and p == 6:
                    emit_setup_b(h + 1, tls[h + 1])
            emit_readout(h, tls[h])
            del tls[h]

    # ======================= AllToAll =======================
    with tc.tile_pool(name="ccs", bufs=8) as ccs:
        for j in range(N_CORES):
            for tt in range(TT4):
                mb0 = (j % 4) * TT4 + tt
                stg = ccs.tile([128, HC], F32, tag="stg")
                nc.vector.tensor_scalar_mul(out=stg, in0=xattn[:, mb0, :],
                                            scalar1=maskbc[:, j:j + 1])
                nc.sync.dma_start(
                    out=cc_in[j * TOK + tt * 128:j * TOK + (tt + 1) * 128, :],
                    in_=stg)
        if fake_cc:
            nc.sync.dma_start(out=cc_out, in_=cc_in)
        else:
            nc.gpsimd.collective_compute(
                "AllToAll", ALU.bypass,
                replica_groups=[list(range(N_CORES))],
                ins=[cc_in.opt()], outs=[cc_out.opt()])

    # ======================= FFN =======================
    with tc.tile_pool(name="ffw", bufs=1) as ffw, \
         tc.tile_pool(name="ffa", bufs=3) as ffa, \
         tc.tile_pool(name="ffs", bufs=4) as ffs, \
         tc.tile_pool(name="w1p", bufs=4) as w1p, \
         tc.tile_pool(name="w2p", bufs=3) as w2p, \
         tc.tile_pool(name="psf", bufs=2, space="PSUM") as psfp, \
         tc.tile_pool(name="pso", bufs=1, space="PSUM") as psop, \
         tc.tile_pool(name="pstf", bufs=2, space="PSUM") as pstf:

        g2bc = ffw.tile([128, D], F32)
        nc.sync.dma_start(out=g2bc, in_=g2_d[None, :].to_broadcast([128, D]))
        be2bc = ffw.tile([128, D], F32)
        nc.sync.dma_start(out=be2bc, in_=be2_d[None, :].to_broadcast([128, D]))
        bf2bc = ffw.tile([128, D], F32)
        nc.sync.dma_start(out=bf2bc, in_=bf2_d[None, :].to_broadcast([128, D]))
        bf1sb = ffw.tile([128, DFF // 128], F32)
        nc.sync.dma_start(out=bf1sb, in_=bf1_d.rearrange("(f p) -> p f", p=128))
        bf1h = ffw.tile([128, DFF // 128], F32)
        nc.scalar.activation(out=bf1h, in_=bf1sb, func=AF.Copy, scale=0.5)
        x1_all = ffw.tile([128, TT4, D], F32)
        xn1T = ffw.tile([128, D // 128, TOK], F32R)
        hT = ffw.tile([128, DFF // 128, TOK], BF16)

        cc_a = ffw.tile([128, TT4, D], F32)
        cc_b = ffw.tile([128, TT4, D], F32)
        for tt in range(TT4):
            for kk in range(4):
                nc.sync.dma_start(out=cc_a[:, tt, kk * HC:(kk + 1) * HC],
                                  in_=cc_out[kk * TOK + tt * 128:
                                             kk * TOK + (tt + 1) * 128, :])
                nc.sync.dma_start(out=cc_b[:, tt, kk * HC:(kk + 1) * HC],
                                  in_=cc_out[(kk + 4) * TOK + tt * 128:
                                             (kk + 4) * TOK + (tt + 1) * 128, :])
        for tt in range(TT4):
            xa = ffa.tile([128, D], F32, tag="xa")
            nc.vector.tensor_add(out=xa, in0=cc_a[:, tt, :], in1=cc_b[:, tt, :])
            xtk = ffa.tile([128, D], F32, tag="xtk")
            nc.sync.dma_start(out=xtk, in_=x_tok[tt * 128:(tt + 1) * 128, :])
            nc.gpsimd.tensor_add(out=x1_all[:, tt, :], in0=xtk, in1=xa)
            # LN2
            st = ffs.tile([128, 2, 6], F32, tag="st")
            for sg in range(2):
                nc.vector.bn_stats(out=st[:, sg, :],
                                   in_=x1_all[:, tt, sg * 512:(sg + 1) * 512])
            mv = ffs.tile([128, 2], F32, tag="mv")
            nc.vector.bn_aggr(out=mv, in_=st)
            rstd = ffs.tile([128, 1], F32, tag="rstd")
            nc.scalar.activation(out=rstd, in_=mv[:, 1:2], func=AF.Sqrt,
                                 bias=epsT, scale=1.0)
            nc.vector.reciprocal(out=rstd, in_=rstd)
            xn1 = ffa.tile([128, D], F32, tag="xn1")
            nc.vector.tensor_scalar(out=xn1, in0=x1_all[:, tt, :],
                                    scalar1=mv[:, 0:1], scalar2=rstd,
                                    op0=ALU.subtract, op1=ALU.mult)
            nc.vector.tensor_mul(out=xn1, in0=xn1, in1=g2bc)
            nc.gpsimd.tensor_add(out=xn1, in0=xn1, in1=be2bc)
            for dd in range(D // 128):
                pt = pstf.tile([128, 128], F32, tag="pt")
                nc.tensor.transpose(pt, xn1[:, dd * 128:(dd + 1) * 128], ident)
                nc.scalar.copy(out=xn1T[:, dd, tt * 128:(tt + 1) * 128], in_=pt)

        # h^T = gelu(W1^T @ xn1^T + bf1)
        for f in range(DFF // 128):
            w1f = w1p.tile([128, D // 128, 128], F32, tag="w1f")
            nc.sync.dma_start(
                out=w1f,
                in_=w1_d.rearrange("(dd p) ff -> p dd ff",
                                   p=128)[:, :, f * 128:(f + 1) * 128])
            w1fr = w1p.tile([128, D // 128, 128], F32R, tag="w1fr")
            nc.gpsimd.tensor_copy(out=w1fr, in_=w1f)
            ph = psfp.tile([128, TOK], F32, tag="ph")
            for dd in range(D // 128):
                nc.tensor.matmul(ph, w1fr[:, dd, :], xn1T[:, dd, :],
                                 start=(dd == 0), stop=(dd == D // 128 - 1))
            # gelu (tanh approx) on y = x/2:
            gy = ffa.tile([128, TOK], F32, tag="gy")
            nc.scalar.activation(out=gy, in_=ph, func=AF.Identity, scale=0.5,
                                 bias=bf1h[:, f:f + 1])
            gt = ffa.tile([128, TOK], F32, tag="gt")
            nc.scalar.activation(out=gt, in_=gy, func=AF.Square, scale=1.0)
            nc.vector.tensor_scalar(out=gt, in0=gt, scalar1=8 * 0.044715 * GC0,
                                    scalar2=2 * GC0, op0=ALU.mult, op1=ALU.add)
            nc.vector.tensor_mul(out=gt, in0=gt, in1=gy)
            nc.scalar.activation(out=gt, in_=gt, func=AF.Tanh, scale=1.0)
            nc.vector.scalar_tensor_tensor(out=hT[:, f, :], in0=gt, scalar=1.0,
                                           in1=gy, op0=ALU.add, op1=ALU.mult)

        # out = x1 + h @ W2 + bf2   (W2 streamed, bf16)
        for dh in range(D // 512):
            pos = [psop.tile([128, 512], F32, tag=f"po{tt}", name=f"po{tt}")
                   for tt in range(TT4)]
            for f in range(DFF // 128):
                w2s = w2p.tile([128, 512], F32, tag="w2s")
                nc.sync.dma_start(out=w2s,
                                  in_=w2_d[f * 128:(f + 1) * 128,
                                           dh * 512:(dh + 1) * 512])
                w2b = w2p.tile([128, 512], BF16, tag="w2b")
                nc.gpsimd.tensor_copy(out=w2b, in_=w2s)
                for tt in range(TT4):
                    nc.tensor.matmul(pos[tt], hT[:, f, tt * 128:(tt + 1) * 128],
                                     w2b, start=(f == 0),
                                     stop=(f == DFF // 128 - 1))
            for tt in range(TT4):
                o1 = ffa.tile([128, 512], F32, tag="o1")
                nc.vector.tensor_add(out=o1, in0=pos[tt],
                                     in1=x1_all[:, tt, dh * 512:(dh + 1) * 512])
                nc.vector.tensor_add(out=o1, in0=o1,
                                     in1=bf2bc[:, dh * 512:(dh + 1) * 512])
                nc.sync.dma_start(out=out_d[tt * 128:(tt + 1) * 128,
                                            dh * 512:(dh + 1) * 512], in_=o1)

    ctx.close()


# ======================= host-side driver =======================

def shard_inputs(inputs, S=S_FULL):
    x = np.ascontiguousarray(inputs["x"], dtype=np.float32)
    TOK = S // 4
    in_maps = []
    for c in range(N_CORES):
        b = c // 4
        hg = c % 4
        hsl = slice(hg * NHL, (hg + 1) * NHL)
        csl = slice(hg * NHL * HD, (hg + 1) * NHL * HD)
        rsl = slice(hg * TOK, (hg + 1) * TOK)
        m = {
            "x_full": x[b],
            "x_heads": x[b][:, csl],
            "x_tok": x[b][rsl, :],
            "wq": inputs["Wq"][hsl].reshape(NHL * HD, HD),
            "wk": inputs["Wk"][hsl].reshape(NHL * HD, HD),
            "wv": inputs["Wv"][hsl].reshape(NHL * HD, HD),
            "wo": inputs["Wo"][hsl].reshape(NHL * HD, HD),
            "omega": inputs["omega"][hsl],
            "g1h": inputs["g1"][csl],
            "be1h": inputs["be1"][csl],
            "g2": inputs["g2"], "be2": inputs["be2"],
            "w1": inputs["W1"], "bf1": inputs["bf1"],
            "w2": inputs["W2"], "bf2": inputs["bf2"],
            "gmask": np.array([1.0 if j // 4 == b else 0.0
                               for j in range(N_CORES)], dtype=np.float32),
        }
        in_maps.append({k: np.ascontiguousarray(v, dtype=np.float32)
                        for k, v in m.items()})
    return in_maps


def assemble_output(results, S=S_FULL):
    TOK = S // 4
    out = np.zeros((B, S, D), dtype=np.float32)
    for c in range(N_CORES):
        b, hg = c // 4, c % 4
        out[b, hg * TOK:(hg + 1) * TOK, :] = results[c]["out"]
    return out


_NC_CACHE = {}


def kernel(**inputs):
    from concourse.bass_utils import run_bass_kernel_spmd
    S = inputs["x"].shape[1]
    if S not in _NC_CACHE:
        _NC_CACHE[S] = build_nc(S)
    nc = _NC_CACHE[S]
    in_maps = shard_inputs(inputs, S)
    res = run_bass_kernel_spmd(nc, in_maps, core_ids=list(range(N_CORES)))
    return assemble_output(res.results, S)
